# revision 41
# baseline (speedup 1.0000x reference)
"""BitFeedForward (ternary-weight SwiGLU-ish FFN) on 8 Trainium2 NeuronCores.

Strategy: data-parallel over tokens (8192 tokens -> 1024/core). Weights are
ternarized on host (exact {-1,0,+1} in bf16); activations are int8-value
quantized on device (integers are exact in bf16), so every matmul runs on the
PE at full bf16 rate and the integer accumulations in fp32 PSUM are exact.

v5 layout: phase B computes u/v FEATURE-major (psum = [128 H, 512 tok]) with
the weight chunk as stationary operand and feature-major quantized activations
as moving operand; h/gh come out feature-major so the mm3 contraction over H
needs NO transposes. Phase B runs token-half-outer (w1/w2 streamed twice,
bandwidth is cheap) so the tail of phase A hides under the first half's
matmuls. Per-token scalars reach the feature-major side via tiny DRAM
scatter+broadcast bounces on the (idle) scalar queue -- never through the
in-order PE queue.

The program is specialized at runtime on two input properties (both true for
the benchmark's setup_inputs, general fallbacks otherwise):
  g12_same: g1 == g2 elementwise -> q1 == q2, quantize once, share q1T.
  g_ones:   g1/g2 all-ones -> skip the x*g multiply; g3 all-ones -> stage h
            directly (no g3 scaling).

  A: per 128-token tile: x -> rmsnorm stats -> int8 quant (token-major) ->
     DRAM; per 512-token half: XBAR-transpose into q1T/q2T + c1/c2 broadcast.
  B: for th in {0,1}: stream w1/w2; mm1+mm2 (N=512), fused silu(c1*u)*(c2*v),
     g3*h staged to DRAM [H,T] f32, running elementwise sum(h^2)/max|g3 h|.
  C: PE-transpose the stat accumulators -> per-token S3/M3 -> c3/rho3.
  D: reload gh chunks [128 H, 1024 tok], quantize in place -> q3T chunks
     (bf16), mm3 over 4 D-quarters x 64 H-chunks into 8 token-tile psums.
"""

import sys

sys.path.insert(0, "/opt/trn_rl_repo")

import numpy as np
import ml_dtypes

import concourse.bass as bass
from concourse import bacc, mybir
from concourse.bass_utils import run_bass_kernel_spmd
from concourse.tile import TileContext, add_dep_helper
from concourse.masks import make_identity

# problem dims
B, S, D, H = 4, 2048, 2048, 8192
NTOK = B * S            # 8192 tokens
NCORES = 8
T_CORE = NTOK // NCORES  # 1024 tokens per core

EPS = 1e-8
C_RINT = float(1.5 * 2.0**23)   # (y + C) - C == rint(y) for |y| < 2^22
ATANH_HALF = float(np.arctanh(np.float64(0.5)))

F32 = mybir.dt.float32
BF16 = mybir.dt.bfloat16

# device loop constants
TT = 8                   # 128-token tiles
TH = 2                   # 512-token halves (phase B moving dim)
HBW = 256                # H columns per w1/w2 stream block
HB = H // HBW            # 32
DC = D // 128            # 16 contraction chunks for mm1/2
HC = H // 128            # 64 contraction chunks for mm3
DQ = 4                   # D quarters for mm3 (512 cols each)


def _build_program(g12_same, g_ones):
    nc = bacc.Bacc("TRN2", target_bir_lowering=False, debug=False)

    x_d = nc.dram_tensor("x", [T_CORE, D], F32, kind="ExternalInput")
    w1_d = nc.dram_tensor("w1q", [D, H], BF16, kind="ExternalInput")
    w2_d = nc.dram_tensor("w2q", [D, H], BF16, kind="ExternalInput")
    w3_d = nc.dram_tensor("w3q", [H, D], BF16, kind="ExternalInput")
    g1_d = nc.dram_tensor("g1", [1, D], F32, kind="ExternalInput")
    g2_d = nc.dram_tensor("g2", [1, D], F32, kind="ExternalInput")
    g3c_d = nc.dram_tensor("g3c", [128, HC], F32, kind="ExternalInput")
    kc_d = nc.dram_tensor("kconst", [1, 3], F32, kind="ExternalInput")
    out_d = nc.dram_tensor("out", [T_CORE, D], F32, kind="ExternalOutput")
    # g3*h staged feature-major: [H, T] f32
    gh_d = nc.dram_tensor("gh_scratch", [H, T_CORE], F32)
    # token-major quantized activations staged for XBAR transpose loads
    q1_d = nc.dram_tensor("q1_scratch", [T_CORE, D], BF16)
    q2_d = q1_d if g12_same else nc.dram_tensor("q2_scratch", [T_CORE, D], BF16)
    # stat bounce buffers for token-broadcast rows (c1 | c2, then rho3)
    c_d = nc.dram_tensor("cstat", [1, 2 * T_CORE], F32)
    c2_d = nc.dram_tensor("cstat_th0", [1, 2 * T_CORE], F32)
    r_d = nc.dram_tensor("rstat", [1, T_CORE], F32)

    w1_r = w1_d.rearrange("(dc p) h -> p dc h", p=128)
    w2_r = w2_d.rearrange("(dc p) h -> p dc h", p=128)

    with TileContext(nc) as tc, bass.ExitStack() as ctx:
        ec = ctx.enter_context
        singles = ec(tc.tile_pool(name="singles", bufs=1))
        parts = ec(tc.tile_pool(name="parts", bufs=4))

        # ---- persistent constants / stats ----
        epst = singles.tile([128, 1], F32, tag="eps")
        nc.vector.memset(epst, EPS)
        ident = singles.tile([128, 128], F32, tag="ident")
        make_identity(nc, ident)
        karep = singles.tile([128, 3], F32, tag="karep")
        nc.sync.dma_start(out=karep, in_=kc_d[:, :].to_broadcast([128, 3]))
        if not g_ones:
            g3c_t = singles.tile([128, HC], F32, tag="g3c")
            nc.sync.dma_start(out=g3c_t, in_=g3c_d[:, :])

        # token-major per-token stats (partition = token within tile tt)
        c3_t = singles.tile([128, TT], F32, tag="c3")
        rho3_t = singles.tile([128, TT], F32, tag="rho3")
        S3_t = singles.tile([128, TT], F32, tag="S3")
        M3_t = singles.tile([128, TT], F32, tag="M3")
        r_t = singles.tile([128, TT], F32, tag="r1")
        # token-broadcast rho3 (used by phase D quantize)
        rho3rep = singles.tile([128, T_CORE], F32, tag="rho3rep")

        def tok_scalars(dst_c, dst_rho, M_ap, r_ap, kcol):
            """denom = max(M*r, 1e-4); dst_c = denom * karep[:,kcol];
            dst_rho = 127 * r / denom."""
            den = parts.tile([128, 1], F32, tag="den")
            nc.vector.tensor_tensor(out=den, in0=M_ap, in1=r_ap,
                                    op=mybir.AluOpType.mult)
            nc.vector.tensor_scalar_max(out=den, in0=den, scalar1=1e-4)
            nc.vector.tensor_scalar(out=dst_c, in0=den,
                                    scalar1=karep[:, kcol:kcol + 1], scalar2=None,
                                    op0=mybir.AluOpType.mult)
            iden = parts.tile([128, 1], F32, tag="iden")
            nc.vector.reciprocal(out=iden, in_=den)
            nc.vector.tensor_tensor(out=iden, in0=iden, in1=r_ap,
                                    op=mybir.AluOpType.mult)
            nc.vector.tensor_scalar(out=dst_rho, in0=iden, scalar1=127.0,
                                    scalar2=None, op0=mybir.AluOpType.mult)

        hbufs = 12 if (g12_same and g_ones) else 8
        with tc.tile_pool(name="bscope", bufs=1) as bsc, \
             tc.tile_pool(name="xpool", bufs=3) as xpool, \
             tc.tile_pool(name="scr", bufs=3 if g12_same else 2) as scr, \
             tc.tile_pool(name="qb", bufs=2) as qb, \
             tc.tile_pool(name="wpool", bufs=3 if g12_same else 2) as wpool, \
             tc.tile_pool(name="hpool", bufs=hbufs) as hpool, \
             tc.tile_pool(name="psB", bufs=7, space="PSUM") as psB:

            if not g_ones:
                g1rep = bsc.tile([128, D], F32, tag="g1rep")
                nc.sync.dma_start(out=g1rep,
                                  in_=g1_d[:, :].to_broadcast([128, D]))
                g2rep = bsc.tile([128, D], F32, tag="g2rep")
                nc.sync.dma_start(out=g2rep,
                                  in_=g2_d[:, :].to_broadcast([128, D]))

            # feature-major activations + per-token c1/c2 rows, one tile per
            # token half so phase B's first half never waits on the second
            q1T = [bsc.tile([128, DC, 512], BF16, tag=f"q1T{th}",
                            name=f"q1T{th}") for th in range(TH)]
            if g12_same:
                q2T = q1T
            else:
                q2T = [bsc.tile([128, DC, 512], BF16, tag=f"q2T{th}",
                                name=f"q2T{th}") for th in range(TH)]
            c1rep = [bsc.tile([128, 512], F32, tag=f"c1rep{th}",
                              name=f"c1rep{th}") for th in range(TH)]
            c2rep = [bsc.tile([128, 512], F32, tag=f"c2rep{th}",
                              name=f"c2rep{th}") for th in range(TH)]
            # c1/c2 token-major stack: cols 0-7 = c1 per tt, 8-15 = c2
            cstack = bsc.tile([128, 16], F32, tag="cstack")
            nc.vector.memset(cstack, 0.0)
            # elementwise stat accumulators [128, T]
            S3run = bsc.tile([128, T_CORE], F32, tag="S3run")
            nc.vector.memset(S3run, 0.0)
            M3run = bsc.tile([128, T_CORE], F32, tag="M3run")
            nc.vector.memset(M3run, 0.0)

            # ======== phase A: x prep -> q1/q2 -> feature-major q1T/q2T
            branches = ((0, q1_d),) if g12_same else ((0, q1_d), (1, q2_d))
            dep_anchor = None
            for tt in range(TT):
                tok0 = tt * 128
                x_t = xpool.tile([128, D], F32, tag="x")
                nc.sync.dma_start(out=x_t, in_=x_d[tok0:tok0 + 128, :])
                sink = scr.tile([128, D], F32, tag="scr")
                ssq = parts.tile([128, 1], F32, tag="ssq")
                nc.scalar.activation(out=sink, in_=x_t,
                                     func=mybir.ActivationFunctionType.Square,
                                     accum_out=ssq)
                # r = 1/sqrt(ssq/D + eps)
                nc.scalar.activation(out=r_t[:, tt:tt + 1], in_=ssq,
                                     func=mybir.ActivationFunctionType.Sqrt,
                                     bias=epst, scale=1.0 / D)
                nc.vector.reciprocal(out=r_t[:, tt:tt + 1], in_=r_t[:, tt:tt + 1])

                for (bi, q_dram) in branches:
                    if g_ones:
                        gx = x_t
                    else:
                        gx = scr.tile([128, D], F32, tag="scr")
                        nc.vector.tensor_tensor(
                            out=gx, in0=x_t, in1=(g1rep if bi == 0 else g2rep),
                            op=mybir.AluOpType.mult)
                    M = parts.tile([128, 1], F32, tag="M")
                    nc.vector.tensor_reduce(out=M, in_=gx,
                                            axis=mybir.AxisListType.X,
                                            op=mybir.AluOpType.max,
                                            apply_absolute_value=True)
                    rho = parts.tile([128, 1], F32, tag="rho")
                    tok_scalars(cstack[:, bi * 8 + tt:bi * 8 + tt + 1], rho,
                                M, r_t[:, tt:tt + 1], bi)
                    if g12_same:
                        # same M, r -> c2 shares denom/rho with c1
                        rho2 = parts.tile([128, 1], F32, tag="rho2",
                                          name=f"rho2_{tt}")
                        tok_scalars(cstack[:, 8 + tt:8 + tt + 1], rho2,
                                    M, r_t[:, tt:tt + 1], 1)
                    # q = rint(gx * rho) via magic constant, cast to bf16
                    y = scr.tile([128, D], F32, tag="scr")
                    nc.scalar.activation(out=y, in_=gx,
                                         func=mybir.ActivationFunctionType.Copy,
                                         scale=rho)
                    qt = qb.tile([128, D], BF16, tag="qb")
                    nc.vector.tensor_scalar(out=qt, in0=y, scalar1=C_RINT,
                                            scalar2=C_RINT,
                                            op0=mybir.AluOpType.add,
                                            op1=mybir.AluOpType.subtract)
                    qst = nc.sync.dma_start(out=q_dram[tok0:tok0 + 128, :],
                                            in_=qt)
                    if tt == 3:
                        dep_anchor = qst

                if tt % 4 == 3:
                    th = tt // 4
                    ts0 = th * 512
                    # feature-major XBAR transpose for this 512-token half
                    xb = nc.sync.dma_start_transpose(
                        q1T[th], q1_d[ts0:ts0 + 512, :])
                    if th == 1:
                        xbar_th1 = xb
                    if not g12_same:
                        xb2 = nc.sync.dma_start_transpose(
                            q2T[th], q2_d[ts0:ts0 + 512, :])
                        if th == 1:
                            xbar2_th1 = xb2
                    if th == 0:
                        # latency-critical: hb0's evac needs c1rep[0] right
                        # after the first psum lands. PE-transpose the stack
                        # (its dep resolves well before the first matmul can
                        # start, so it doesn't block the in-order PE queue)
                        # and bounce a contiguous row through DRAM.
                        cps0 = psB.tile([128, 128], F32, tag="ps",
                                        name="cps0")
                        nc.tensor.transpose(cps0[0:16, :], cstack, ident)
                        csb0 = parts.tile([16, 128], F32, tag="csb0")
                        nc.vector.tensor_copy(out=csb0, in_=cps0[0:16, :])
                        nc.scalar.dma_start(
                            out=c2_d.rearrange("one (a b) -> a (one b)", a=16),
                            in_=csb0)
                        nc.scalar.dma_start(
                            out=c1rep[0],
                            in_=c2_d[0:1, 0:512].to_broadcast([128, 512]))
                        nc.scalar.dma_start(
                            out=c2rep[0],
                            in_=c2_d[0:1, T_CORE:T_CORE + 512].to_broadcast(
                                [128, 512]))
                    else:
                        # th1's rows are needed ~400us later; the cheap
                        # scatter bounce off the compute path suffices
                        for bi, crep in ((0, c1rep), (1, c2rep)):
                            cs = bi * T_CORE + ts0
                            nc.scalar.dma_start(
                                out=c_d[0:1, cs:cs + 512].rearrange(
                                    "one (a b) -> b (one a)", a=4),
                                in_=cstack[:, bi * 8 + th * 4:
                                           bi * 8 + th * 4 + 4])
                            nc.scalar.dma_start(
                                out=crep[th],
                                in_=c_d[0:1, cs:cs + 512].to_broadcast(
                                    [128, 512]))

            # ======== phase C helper: per-token-tile stat finalization
            def stats_for_tt(tt):
                tok0 = tt * 128
                pts = psB.tile([128, 128], F32, tag="ps", name=f"ptS{tt}")
                nc.tensor.transpose(pts, S3run[:, tok0:tok0 + 128], ident)
                nc.vector.tensor_reduce(out=S3_t[:, tt:tt + 1], in_=pts,
                                        axis=mybir.AxisListType.X,
                                        op=mybir.AluOpType.add)
                ptm = psB.tile([128, 128], F32, tag="ps", name=f"ptM{tt}")
                nc.tensor.transpose(ptm, M3run[:, tok0:tok0 + 128], ident)
                nc.vector.tensor_reduce(out=M3_t[:, tt:tt + 1], in_=ptm,
                                        axis=mybir.AxisListType.X,
                                        op=mybir.AluOpType.max)
                r3 = parts.tile([128, 1], F32, tag="r3", name=f"r3_{tt}")
                nc.scalar.activation(out=r3, in_=S3_t[:, tt:tt + 1],
                                     func=mybir.ActivationFunctionType.Sqrt,
                                     bias=epst, scale=1.0 / H)
                nc.vector.reciprocal(out=r3, in_=r3)
                tok_scalars(c3_t[:, tt:tt + 1], rho3_t[:, tt:tt + 1],
                            M3_t[:, tt:tt + 1], r3, 2)

            # ======== phase B: mm1/mm2 feature-major + h + stats
            for th in range(TH):
                ts = slice(th * 512, (th + 1) * 512)
                for hb in range(HB):
                    w1b = wpool.tile([128, DC, HBW], BF16, tag="w1b")
                    wl1 = nc.sync.dma_start(
                        out=w1b, in_=w1_r[:, :, hb * HBW:(hb + 1) * HBW])
                    w2b = wpool.tile([128, DC, HBW], BF16, tag="w2b")
                    wl2 = nc.sync.dma_start(
                        out=w2b, in_=w2_r[:, :, hb * HBW:(hb + 1) * HBW])
                    if th == 0 and hb < 4 and dep_anchor is not None:
                        # keep the (dep-free) bulk weight prefetch from being
                        # hoisted ahead of phase A's latency-critical DMAs
                        add_dep_helper(wl1.ins, dep_anchor.ins, sync=False,
                                       reason="defer w prefetch")
                        add_dep_helper(wl2.ins, dep_anchor.ins, sync=False,
                                       reason="defer w prefetch")
                        # ...but ahead of the th1 XBAR, whose issue blocks the
                        # sync queue for its whole ~9.5us transfer and would
                        # starve hb1-3's weights (th1's data isn't needed for
                        # another ~400us)
                        add_dep_helper(xbar_th1.ins, wl2.ins, sync=False,
                                       reason="xbar th1 after early w loads")
                        if not g12_same:
                            add_dep_helper(xbar2_th1.ins, wl2.ins, sync=False,
                                           reason="xbar2 th1 after early w")
                    for hc2 in range(2):
                        ghc = hb * 2 + hc2
                        pu = psB.tile([128, 512], F32, tag="ps")
                        for dc in range(DC):
                            nc.tensor.matmul(
                                pu,
                                lhsT=w1b[:, dc, hc2 * 128:(hc2 + 1) * 128],
                                rhs=q1T[th][:, dc, :],
                                start=(dc == 0), stop=(dc == DC - 1))
                        pv = psB.tile([128, 512], F32, tag="ps")
                        for dc in range(DC):
                            nc.tensor.matmul(
                                pv,
                                lhsT=w2b[:, dc, hc2 * 128:(hc2 + 1) * 128],
                                rhs=q2T[th][:, dc, :],
                                start=(dc == 0), stop=(dc == DC - 1))
                        # h = silu(c1*u) * (c2*v), all [128 H, 512 tok]
                        t1 = hpool.tile([128, 512], F32, tag="h")
                        nc.vector.tensor_tensor(out=t1, in0=pu, in1=c1rep[th],
                                                op=mybir.AluOpType.mult)
                        sg = hpool.tile([128, 512], F32, tag="h")
                        nc.scalar.activation(
                            out=sg, in_=t1,
                            func=mybir.ActivationFunctionType.Sigmoid)
                        sw = hpool.tile([128, 512], F32, tag="h")
                        nc.vector.tensor_tensor(out=sw, in0=sg, in1=t1,
                                                op=mybir.AluOpType.mult)
                        t2 = hpool.tile([128, 512], F32, tag="h")
                        nc.vector.tensor_tensor(out=t2, in0=pv, in1=c2rep[th],
                                                op=mybir.AluOpType.mult)
                        ht = hpool.tile([128, 512], F32, tag="h")
                        nc.vector.tensor_tensor(out=ht, in0=sw, in1=t2,
                                                op=mybir.AluOpType.mult)
                        # stats accumulate
                        hsq = hpool.tile([128, 512], F32, tag="h")
                        nc.scalar.activation(
                            out=hsq, in_=ht,
                            func=mybir.ActivationFunctionType.Square)
                        nc.vector.tensor_tensor(out=S3run[:, ts],
                                                in0=S3run[:, ts],
                                                in1=hsq, op=mybir.AluOpType.add)
                        if g_ones:
                            gh = ht
                            gha = hpool.tile([128, 512], F32, tag="h")
                            nc.scalar.activation(
                                out=gha, in_=ht,
                                func=mybir.ActivationFunctionType.Abs)
                        else:
                            gh = hpool.tile([128, 512], F32, tag="h")
                            nc.scalar.activation(
                                out=gh, in_=ht,
                                func=mybir.ActivationFunctionType.Copy,
                                scale=g3c_t[:, ghc:ghc + 1])
                            gha = hpool.tile([128, 512], F32, tag="h")
                            nc.scalar.activation(
                                out=gha, in_=ht,
                                func=mybir.ActivationFunctionType.Abs,
                                scale=g3c_t[:, ghc:ghc + 1])
                        nc.vector.tensor_tensor(out=M3run[:, ts],
                                                in0=M3run[:, ts],
                                                in1=gha, op=mybir.AluOpType.max)
                        nc.sync.dma_start(
                            out=gh_d[ghc * 128:(ghc + 1) * 128, ts], in_=gh)
                    if th == 1 and hb == 0:
                        # th0's stat columns are final; emitting the PE-queue
                        # transposes after th1's first block keeps them off
                        # the th-boundary critical path
                        for tt in range(4):
                            stats_for_tt(tt)
                if th == 1:
                    for tt in range(4, 8):
                        stats_for_tt(tt)

            # ======== phase C: rho3 -> token-broadcast (scalar queue)
            nc.scalar.dma_start(
                out=r_d[0:1, :].rearrange("one (a b) -> b (one a)", a=TT),
                in_=rho3_t)
            nc.scalar.dma_start(out=rho3rep,
                                in_=r_d[:, :].to_broadcast([128, T_CORE]))

        # ======== phase D: quantize q3T chunks + mm3 (no transposes)
        with tc.tile_pool(name="q3p", bufs=1) as q3p, \
             tc.tile_pool(name="ghl", bufs=8) as ghlp, \
             tc.tile_pool(name="w3p", bufs=6) as w3p, \
             tc.tile_pool(name="outp", bufs=4) as outp, \
             tc.tile_pool(name="psD", bufs=8, space="PSUM") as psD:

            # gh chunk loads only depend on phase-B stores; with bufs=8 the
            # first loads complete during phase B/C, so quantize fires the
            # moment rho3rep lands and mm3 ramps without DMA stalls.
            q3c = []
            for hc in range(HC):
                ghl = ghlp.tile([128, T_CORE], F32, tag="ghl")
                nc.scalar.dma_start(out=ghl,
                                    in_=gh_d[hc * 128:(hc + 1) * 128, :])
                nc.vector.tensor_tensor(out=ghl, in0=ghl, in1=rho3rep,
                                        op=mybir.AluOpType.mult)
                q3 = q3p.tile([128, T_CORE], BF16, tag=f"q3_{hc}")
                nc.vector.tensor_scalar(out=q3, in0=ghl, scalar1=C_RINT,
                                        scalar2=C_RINT,
                                        op0=mybir.AluOpType.add,
                                        op1=mybir.AluOpType.subtract)
                q3c.append(q3)

            for dq in range(DQ):
                dcol = dq * 512
                pos = [psD.tile([128, 512], F32, tag="po", name=f"po{dq}_{t}")
                       for t in range(TT)]
                for hc in range(HC):
                    w3b = w3p.tile([128, 512], BF16, tag="w3b")
                    nc.sync.dma_start(
                        out=w3b,
                        in_=w3_d[hc * 128:(hc + 1) * 128, dcol:dcol + 512])
                    for t in range(TT):
                        nc.tensor.matmul(
                            pos[t],
                            lhsT=q3c[hc][:, t * 128:(t + 1) * 128],
                            rhs=w3b,
                            start=(hc == 0), stop=(hc == HC - 1),
                            skip_group_check=True)
                for t in range(TT):
                    ob = outp.tile([128, 512], F32, tag="ob")
                    if t % 2 == 0:
                        nc.scalar.mul(out=ob, in_=pos[t], mul=c3_t[:, t:t + 1])
                    else:
                        nc.vector.tensor_scalar(out=ob, in0=pos[t],
                                                scalar1=c3_t[:, t:t + 1],
                                                scalar2=None,
                                                op0=mybir.AluOpType.mult)
                    nc.scalar.dma_start(
                        out=out_d[t * 128:(t + 1) * 128, dcol:dcol + 512],
                        in_=ob)

    nc.compile()
    return nc


_NC_CACHE = {}


def _get_program(g12_same=True, g_ones=True):
    key = (bool(g12_same), bool(g_ones))
    if key not in _NC_CACHE:
        _NC_CACHE[key] = _build_program(*key)
    return _NC_CACHE[key]


def _ternary_T(w):
    """Host ternarization matching round(tanh(w/(mean|w|+eps))) in value.
    Uses CPU-jax to replicate the reference's fp32 tanh bit-for-bit.
    Returns (transposed ternary bf16 array, arctanh(s) as float32)."""
    w32 = np.asarray(w, dtype=np.float32)
    try:
        import jax
        import jax.numpy as jnp
        cpu = jax.devices("cpu")[0]
        with jax.default_device(cpu):
            s = jnp.mean(jnp.abs(jnp.asarray(w32)))
            t = np.asarray(jnp.round(jnp.tanh(w32 / (s + np.float32(EPS)))))
            a = np.float32(jnp.arctanh(s))
    except Exception:
        s32 = np.float32(np.mean(np.abs(w32), dtype=np.float64))
        denom = np.float32(s32 + np.float32(EPS))
        thresh = np.float32(ATANH_HALF) * denom
        t = np.sign(w32) * (np.abs(w32) > thresh)
        a = np.float32(np.arctanh(np.float64(s32)))
    return np.ascontiguousarray(t.T).astype(ml_dtypes.bfloat16), a


def _gflags(g1, g2, g3):
    g1f = np.asarray(g1, np.float32).reshape(-1)
    g2f = np.asarray(g2, np.float32).reshape(-1)
    g3f = np.asarray(g3, np.float32).reshape(-1)
    g12_same = bool(np.array_equal(g1f, g2f))
    g_ones = bool((g1f == 1.0).all() and (g2f == 1.0).all()
                  and (g3f == 1.0).all())
    return g12_same, g_ones


def _make_inputs(x, w1, g1, w2, g2, w3, g3):
    x32 = np.asarray(x, np.float32).reshape(NTOK, D)
    w1q, a1 = _ternary_T(w1)            # [D, H]
    w2q, a2 = _ternary_T(w2)            # [D, H]
    w3q, a3 = _ternary_T(w3)            # [H, D] (w3 is [D, H])
    g1f = np.ascontiguousarray(np.asarray(g1, np.float32).reshape(1, D))
    g2f = np.ascontiguousarray(np.asarray(g2, np.float32).reshape(1, D))
    g3c = np.ascontiguousarray(
        np.asarray(g3, np.float32).reshape(HC, 128).T)   # [128, HC]
    kconst = np.array([[a1 / 127.0, a2 / 127.0, a3 / 127.0]], np.float32)
    in_maps = []
    for c in range(NCORES):
        in_maps.append({
            "x": np.ascontiguousarray(x32[c * T_CORE:(c + 1) * T_CORE]),
            "w1q": w1q, "w2q": w2q, "w3q": w3q,
            "g1": g1f, "g2": g2f, "g3c": g3c,
            "kconst": kconst,
        })
    return in_maps


def kernel(x, w1, g1, w2, g2, w3, g3):
    g12_same, g_ones = _gflags(g1, g2, g3)
    nc = _get_program(g12_same, g_ones)
    in_maps = _make_inputs(x, w1, g1, w2, g2, w3, g3)
    res = run_bass_kernel_spmd(nc, in_maps, list(range(NCORES)))
    out = np.concatenate([res.results[c]["out"] for c in range(NCORES)], axis=0)
    return out.reshape(B, S, D)


# revision 45
# speedup vs baseline: 1.0118x; 1.0118x over previous
"""BitFeedForward (ternary-weight SwiGLU-ish FFN) on 8 Trainium2 NeuronCores.

Strategy: data-parallel over tokens (8192 tokens -> 1024/core). Weights are
ternarized on host (exact {-1,0,+1} in bf16); activations are int8-value
quantized on device (integers are exact in bf16), so every matmul runs on the
PE at full bf16 rate and the integer accumulations in fp32 PSUM are exact.

v5 layout: phase B computes u/v FEATURE-major (psum = [128 H, 512 tok]) with
the weight chunk as stationary operand and feature-major quantized activations
as moving operand; h/gh come out feature-major so the mm3 contraction over H
needs NO transposes. Phase B runs token-half-outer (w1/w2 streamed twice,
bandwidth is cheap) so the tail of phase A hides under the first half's
matmuls. Per-token scalars reach the feature-major side via tiny DRAM
scatter+broadcast bounces on the (idle) scalar queue -- never through the
in-order PE queue.

The program is specialized at runtime on two input properties (both true for
the benchmark's setup_inputs, general fallbacks otherwise):
  g12_same: g1 == g2 elementwise -> q1 == q2, quantize once, share q1T.
  g_ones:   g1/g2 all-ones -> skip the x*g multiply; g3 all-ones -> stage h
            directly (no g3 scaling).

  A: per 128-token tile: x -> rmsnorm stats -> int8 quant (token-major) ->
     DRAM; per 512-token half: XBAR-transpose into q1T/q2T + c1/c2 broadcast.
  B: for th in {0,1}: stream w1/w2; mm1+mm2 (N=512), fused silu(c1*u)*(c2*v),
     g3*h staged to DRAM [H,T] f32, running elementwise sum(h^2)/max|g3 h|.
  C: PE-transpose the stat accumulators -> per-token S3/M3 -> c3/rho3.
  D: reload gh chunks [128 H, 1024 tok], quantize in place -> q3T chunks
     (bf16), mm3 over 4 D-quarters x 64 H-chunks into 8 token-tile psums.
"""

import sys

sys.path.insert(0, "/opt/trn_rl_repo")

import numpy as np
import ml_dtypes

import concourse.bass as bass
from concourse import bacc, mybir
from concourse.bass_utils import run_bass_kernel_spmd
from concourse.tile import TileContext, add_dep_helper
from concourse.masks import make_identity

# problem dims
B, S, D, H = 4, 2048, 2048, 8192
NTOK = B * S            # 8192 tokens
NCORES = 8
T_CORE = NTOK // NCORES  # 1024 tokens per core

EPS = 1e-8
C_RINT = float(1.5 * 2.0**23)   # (y + C) - C == rint(y) for |y| < 2^22
ATANH_HALF = float(np.arctanh(np.float64(0.5)))

F32 = mybir.dt.float32
BF16 = mybir.dt.bfloat16

# device loop constants
TT = 8                   # 128-token tiles
TH = 2                   # 512-token halves (phase B moving dim)
HBW = 256                # H columns per w1/w2 stream block
HB = H // HBW            # 32
DC = D // 128            # 16 contraction chunks for mm1/2
HC = H // 128            # 64 contraction chunks for mm3
DQ = 4                   # D quarters for mm3 (512 cols each)


def _build_program(g12_same, g_ones):
    nc = bacc.Bacc("TRN2", target_bir_lowering=False, debug=False)

    x_d = nc.dram_tensor("x", [T_CORE, D], F32, kind="ExternalInput")
    w1_d = nc.dram_tensor("w1q", [D, H], BF16, kind="ExternalInput")
    w2_d = nc.dram_tensor("w2q", [D, H], BF16, kind="ExternalInput")
    w3_d = nc.dram_tensor("w3q", [H, D], BF16, kind="ExternalInput")
    g1_d = nc.dram_tensor("g1", [1, D], F32, kind="ExternalInput")
    g2_d = nc.dram_tensor("g2", [1, D], F32, kind="ExternalInput")
    g3c_d = nc.dram_tensor("g3c", [128, HC], F32, kind="ExternalInput")
    kc_d = nc.dram_tensor("kconst", [1, 3], F32, kind="ExternalInput")
    out_d = nc.dram_tensor("out", [T_CORE, D], F32, kind="ExternalOutput")
    # g3*h staged feature-major: [H, T] f32
    gh_d = nc.dram_tensor("gh_scratch", [H, T_CORE], F32)
    # token-major quantized activations staged for XBAR transpose loads
    q1_d = nc.dram_tensor("q1_scratch", [T_CORE, D], BF16)
    q2_d = q1_d if g12_same else nc.dram_tensor("q2_scratch", [T_CORE, D], BF16)
    # stat bounce buffers for token-broadcast rows (c1 | c2, then rho3)
    c_d = nc.dram_tensor("cstat", [1, 2 * T_CORE], F32)
    c2_d = nc.dram_tensor("cstat_th0", [1, 2 * T_CORE], F32)
    r_d = nc.dram_tensor("rstat", [1, T_CORE], F32)

    w1_r = w1_d.rearrange("(dc p) h -> p dc h", p=128)
    w2_r = w2_d.rearrange("(dc p) h -> p dc h", p=128)

    with TileContext(nc) as tc, bass.ExitStack() as ctx:
        ec = ctx.enter_context
        singles = ec(tc.tile_pool(name="singles", bufs=1))
        parts = ec(tc.tile_pool(name="parts", bufs=4))

        # ---- persistent constants / stats ----
        epst = singles.tile([128, 1], F32, tag="eps")
        nc.vector.memset(epst, EPS)
        ident = singles.tile([128, 128], F32, tag="ident")
        make_identity(nc, ident)
        karep = singles.tile([128, 3], F32, tag="karep")
        nc.sync.dma_start(out=karep, in_=kc_d[:, :].to_broadcast([128, 3]))
        if not g_ones:
            g3c_t = singles.tile([128, HC], F32, tag="g3c")
            nc.sync.dma_start(out=g3c_t, in_=g3c_d[:, :])

        # token-major per-token stats (partition = token within tile tt)
        c3_t = singles.tile([128, TT], F32, tag="c3")
        rho3_t = singles.tile([128, TT], F32, tag="rho3")
        S3_t = singles.tile([128, TT], F32, tag="S3")
        M3_t = singles.tile([128, TT], F32, tag="M3")
        r_t = singles.tile([128, TT], F32, tag="r1")
        # token-broadcast rho3 (used by phase D quantize)
        rho3rep = singles.tile([128, T_CORE], F32, tag="rho3rep")

        def tok_scalars(dst_c, dst_rho, M_ap, r_ap, kcol):
            """denom = max(M*r, 1e-4); dst_c = denom * karep[:,kcol];
            dst_rho = 127 * r / denom."""
            den = parts.tile([128, 1], F32, tag="den")
            nc.vector.tensor_tensor(out=den, in0=M_ap, in1=r_ap,
                                    op=mybir.AluOpType.mult)
            nc.vector.tensor_scalar_max(out=den, in0=den, scalar1=1e-4)
            nc.vector.tensor_scalar(out=dst_c, in0=den,
                                    scalar1=karep[:, kcol:kcol + 1], scalar2=None,
                                    op0=mybir.AluOpType.mult)
            iden = parts.tile([128, 1], F32, tag="iden")
            nc.vector.reciprocal(out=iden, in_=den)
            nc.vector.tensor_tensor(out=iden, in0=iden, in1=r_ap,
                                    op=mybir.AluOpType.mult)
            nc.vector.tensor_scalar(out=dst_rho, in0=iden, scalar1=127.0,
                                    scalar2=None, op0=mybir.AluOpType.mult)

        hbufs = 12 if (g12_same and g_ones) else 8
        with tc.tile_pool(name="bscope", bufs=1) as bsc, \
             tc.tile_pool(name="xpool", bufs=3) as xpool, \
             tc.tile_pool(name="scr", bufs=3 if g12_same else 2) as scr, \
             tc.tile_pool(name="qb", bufs=2) as qb, \
             tc.tile_pool(name="wpool", bufs=3 if g12_same else 2) as wpool, \
             tc.tile_pool(name="hpool", bufs=hbufs) as hpool, \
             tc.tile_pool(name="psB", bufs=7, space="PSUM") as psB:

            if not g_ones:
                g1rep = bsc.tile([128, D], F32, tag="g1rep")
                nc.sync.dma_start(out=g1rep,
                                  in_=g1_d[:, :].to_broadcast([128, D]))
                g2rep = bsc.tile([128, D], F32, tag="g2rep")
                nc.sync.dma_start(out=g2rep,
                                  in_=g2_d[:, :].to_broadcast([128, D]))

            # feature-major activations + per-token c1/c2 rows, one tile per
            # token half so phase B's first half never waits on the second
            q1T = [bsc.tile([128, DC, 512], BF16, tag=f"q1T{th}",
                            name=f"q1T{th}") for th in range(TH)]
            if g12_same:
                q2T = q1T
            else:
                q2T = [bsc.tile([128, DC, 512], BF16, tag=f"q2T{th}",
                                name=f"q2T{th}") for th in range(TH)]
            c1rep = [bsc.tile([128, 512], F32, tag=f"c1rep{th}",
                              name=f"c1rep{th}") for th in range(TH)]
            c2rep = [bsc.tile([128, 512], F32, tag=f"c2rep{th}",
                              name=f"c2rep{th}") for th in range(TH)]
            # c1/c2 token-major stack: cols 0-7 = c1 per tt, 8-15 = c2
            cstack = bsc.tile([128, 16], F32, tag="cstack")
            nc.vector.memset(cstack, 0.0)
            # elementwise stat accumulators [128, T]
            S3run = bsc.tile([128, T_CORE], F32, tag="S3run")
            nc.vector.memset(S3run, 0.0)
            M3run = bsc.tile([128, T_CORE], F32, tag="M3run")
            nc.vector.memset(M3run, 0.0)

            # ======== phase A: x prep -> q1/q2 -> feature-major q1T/q2T
            branches = ((0, q1_d),) if g12_same else ((0, q1_d), (1, q2_d))
            dep_anchor = None
            for tt in range(TT):
                tok0 = tt * 128
                x_t = xpool.tile([128, D], F32, tag="x")
                xld = nc.sync.dma_start(out=x_t, in_=x_d[tok0:tok0 + 128, :])
                if tt == 3:
                    dep_anchor = xld
                sink = scr.tile([128, D], F32, tag="scr")
                ssq = parts.tile([128, 1], F32, tag="ssq")
                nc.scalar.activation(out=sink, in_=x_t,
                                     func=mybir.ActivationFunctionType.Square,
                                     accum_out=ssq)
                # r = 1/sqrt(ssq/D + eps)
                nc.scalar.activation(out=r_t[:, tt:tt + 1], in_=ssq,
                                     func=mybir.ActivationFunctionType.Sqrt,
                                     bias=epst, scale=1.0 / D)
                nc.vector.reciprocal(out=r_t[:, tt:tt + 1], in_=r_t[:, tt:tt + 1])

                for (bi, q_dram) in branches:
                    if g_ones:
                        gx = x_t
                    else:
                        gx = scr.tile([128, D], F32, tag="scr")
                        nc.vector.tensor_tensor(
                            out=gx, in0=x_t, in1=(g1rep if bi == 0 else g2rep),
                            op=mybir.AluOpType.mult)
                    M = parts.tile([128, 1], F32, tag="M")
                    nc.vector.tensor_reduce(out=M, in_=gx,
                                            axis=mybir.AxisListType.X,
                                            op=mybir.AluOpType.max,
                                            apply_absolute_value=True)
                    rho = parts.tile([128, 1], F32, tag="rho")
                    tok_scalars(cstack[:, bi * 8 + tt:bi * 8 + tt + 1], rho,
                                M, r_t[:, tt:tt + 1], bi)
                    if g12_same:
                        # same M, r -> c2 shares denom/rho with c1
                        rho2 = parts.tile([128, 1], F32, tag="rho2",
                                          name=f"rho2_{tt}")
                        tok_scalars(cstack[:, 8 + tt:8 + tt + 1], rho2,
                                    M, r_t[:, tt:tt + 1], 1)
                    # q = rint(gx * rho) via magic constant, cast to bf16
                    y = scr.tile([128, D], F32, tag="scr")
                    nc.scalar.activation(out=y, in_=gx,
                                         func=mybir.ActivationFunctionType.Copy,
                                         scale=rho)
                    qt = qb.tile([128, D], BF16, tag="qb")
                    nc.vector.tensor_scalar(out=qt, in0=y, scalar1=C_RINT,
                                            scalar2=C_RINT,
                                            op0=mybir.AluOpType.add,
                                            op1=mybir.AluOpType.subtract)
                    nc.sync.dma_start(out=q_dram[tok0:tok0 + 128, :], in_=qt)

                if tt % 4 == 3:
                    th = tt // 4
                    ts0 = th * 512
                    # feature-major XBAR transpose for this 512-token half
                    nc.sync.dma_start_transpose(
                        q1T[th], q1_d[ts0:ts0 + 512, :])
                    if not g12_same:
                        nc.sync.dma_start_transpose(
                            q2T[th], q2_d[ts0:ts0 + 512, :])
                    if th == 0:
                        # latency-critical: hb0's evac needs c1rep[0] right
                        # after the first psum lands. PE-transpose the stack
                        # (its dep resolves well before the first matmul can
                        # start, so it doesn't block the in-order PE queue)
                        # and bounce a contiguous row through DRAM.
                        cps0 = psB.tile([128, 128], F32, tag="ps",
                                        name="cps0")
                        nc.tensor.transpose(cps0[0:16, :], cstack, ident)
                        csb0 = parts.tile([16, 128], F32, tag="csb0")
                        nc.vector.tensor_copy(out=csb0, in_=cps0[0:16, :])
                        nc.scalar.dma_start(
                            out=c2_d.rearrange("one (a b) -> a (one b)", a=16),
                            in_=csb0)
                        nc.scalar.dma_start(
                            out=c1rep[0],
                            in_=c2_d[0:1, 0:512].to_broadcast([128, 512]))
                        nc.scalar.dma_start(
                            out=c2rep[0],
                            in_=c2_d[0:1, T_CORE:T_CORE + 512].to_broadcast(
                                [128, 512]))
                    else:
                        # th1's rows are needed ~400us later; the cheap
                        # scatter bounce off the compute path suffices
                        for bi, crep in ((0, c1rep), (1, c2rep)):
                            cs = bi * T_CORE + ts0
                            nc.scalar.dma_start(
                                out=c_d[0:1, cs:cs + 512].rearrange(
                                    "one (a b) -> b (one a)", a=4),
                                in_=cstack[:, bi * 8 + th * 4:
                                           bi * 8 + th * 4 + 4])
                            nc.scalar.dma_start(
                                out=crep[th],
                                in_=c_d[0:1, cs:cs + 512].to_broadcast(
                                    [128, 512]))

            # ======== phase C helper: per-token-tile stat finalization
            def stats_for_tt(tt):
                tok0 = tt * 128
                pts = psB.tile([128, 128], F32, tag="ps", name=f"ptS{tt}")
                nc.tensor.transpose(pts, S3run[:, tok0:tok0 + 128], ident)
                nc.vector.tensor_reduce(out=S3_t[:, tt:tt + 1], in_=pts,
                                        axis=mybir.AxisListType.X,
                                        op=mybir.AluOpType.add)
                ptm = psB.tile([128, 128], F32, tag="ps", name=f"ptM{tt}")
                nc.tensor.transpose(ptm, M3run[:, tok0:tok0 + 128], ident)
                nc.vector.tensor_reduce(out=M3_t[:, tt:tt + 1], in_=ptm,
                                        axis=mybir.AxisListType.X,
                                        op=mybir.AluOpType.max)
                r3 = parts.tile([128, 1], F32, tag="r3", name=f"r3_{tt}")
                nc.scalar.activation(out=r3, in_=S3_t[:, tt:tt + 1],
                                     func=mybir.ActivationFunctionType.Sqrt,
                                     bias=epst, scale=1.0 / H)
                nc.vector.reciprocal(out=r3, in_=r3)
                tok_scalars(c3_t[:, tt:tt + 1], rho3_t[:, tt:tt + 1],
                            M3_t[:, tt:tt + 1], r3, 2)

            # ======== phase B: mm1/mm2 feature-major + h + stats
            for th in range(TH):
                ts = slice(th * 512, (th + 1) * 512)
                for hb in range(HB):
                    w1b = wpool.tile([128, DC, HBW], BF16, tag="w1b")
                    wl1 = nc.sync.dma_start(
                        out=w1b, in_=w1_r[:, :, hb * HBW:(hb + 1) * HBW])
                    w2b = wpool.tile([128, DC, HBW], BF16, tag="w2b")
                    wl2 = nc.sync.dma_start(
                        out=w2b, in_=w2_r[:, :, hb * HBW:(hb + 1) * HBW])
                    if th == 0 and hb < 4 and dep_anchor is not None:
                        # keep the (dep-free) bulk weight prefetch from being
                        # hoisted ahead of phase A's latency-critical DMAs
                        add_dep_helper(wl1.ins, dep_anchor.ins, sync=False,
                                       reason="defer w prefetch")
                        add_dep_helper(wl2.ins, dep_anchor.ins, sync=False,
                                       reason="defer w prefetch")
                    for hc2 in range(2):
                        ghc = hb * 2 + hc2
                        pu = psB.tile([128, 512], F32, tag="ps")
                        for dc in range(DC):
                            nc.tensor.matmul(
                                pu,
                                lhsT=w1b[:, dc, hc2 * 128:(hc2 + 1) * 128],
                                rhs=q1T[th][:, dc, :],
                                start=(dc == 0), stop=(dc == DC - 1))
                        pv = psB.tile([128, 512], F32, tag="ps")
                        for dc in range(DC):
                            nc.tensor.matmul(
                                pv,
                                lhsT=w2b[:, dc, hc2 * 128:(hc2 + 1) * 128],
                                rhs=q2T[th][:, dc, :],
                                start=(dc == 0), stop=(dc == DC - 1))
                        # h = silu(c1*u) * (c2*v), all [128 H, 512 tok]
                        t1 = hpool.tile([128, 512], F32, tag="h")
                        nc.vector.tensor_tensor(out=t1, in0=pu, in1=c1rep[th],
                                                op=mybir.AluOpType.mult)
                        sg = hpool.tile([128, 512], F32, tag="h")
                        nc.scalar.activation(
                            out=sg, in_=t1,
                            func=mybir.ActivationFunctionType.Sigmoid)
                        sw = hpool.tile([128, 512], F32, tag="h")
                        nc.vector.tensor_tensor(out=sw, in0=sg, in1=t1,
                                                op=mybir.AluOpType.mult)
                        t2 = hpool.tile([128, 512], F32, tag="h")
                        nc.vector.tensor_tensor(out=t2, in0=pv, in1=c2rep[th],
                                                op=mybir.AluOpType.mult)
                        ht = hpool.tile([128, 512], F32, tag="h")
                        nc.vector.tensor_tensor(out=ht, in0=sw, in1=t2,
                                                op=mybir.AluOpType.mult)
                        # stats accumulate
                        hsq = hpool.tile([128, 512], F32, tag="h")
                        nc.scalar.activation(
                            out=hsq, in_=ht,
                            func=mybir.ActivationFunctionType.Square)
                        nc.vector.tensor_tensor(out=S3run[:, ts],
                                                in0=S3run[:, ts],
                                                in1=hsq, op=mybir.AluOpType.add)
                        if g_ones:
                            gh = ht
                            gha = hpool.tile([128, 512], F32, tag="h")
                            nc.scalar.activation(
                                out=gha, in_=ht,
                                func=mybir.ActivationFunctionType.Abs)
                        else:
                            gh = hpool.tile([128, 512], F32, tag="h")
                            nc.scalar.activation(
                                out=gh, in_=ht,
                                func=mybir.ActivationFunctionType.Copy,
                                scale=g3c_t[:, ghc:ghc + 1])
                            gha = hpool.tile([128, 512], F32, tag="h")
                            nc.scalar.activation(
                                out=gha, in_=ht,
                                func=mybir.ActivationFunctionType.Abs,
                                scale=g3c_t[:, ghc:ghc + 1])
                        nc.vector.tensor_tensor(out=M3run[:, ts],
                                                in0=M3run[:, ts],
                                                in1=gha, op=mybir.AluOpType.max)
                        nc.sync.dma_start(
                            out=gh_d[ghc * 128:(ghc + 1) * 128, ts], in_=gh)
                    if th == 1 and hb == 0:
                        # th0's stat columns are final; emitting the PE-queue
                        # transposes after th1's first block keeps them off
                        # the th-boundary critical path
                        for tt in range(4):
                            stats_for_tt(tt)
                if th == 1:
                    for tt in range(4, 8):
                        stats_for_tt(tt)

            # ======== phase C: rho3 -> token-broadcast (scalar queue)
            nc.scalar.dma_start(
                out=r_d[0:1, :].rearrange("one (a b) -> b (one a)", a=TT),
                in_=rho3_t)
            nc.scalar.dma_start(out=rho3rep,
                                in_=r_d[:, :].to_broadcast([128, T_CORE]))

        # ======== phase D: quantize q3T chunks + mm3 (no transposes)
        with tc.tile_pool(name="q3p", bufs=1) as q3p, \
             tc.tile_pool(name="ghl", bufs=8) as ghlp, \
             tc.tile_pool(name="w3p", bufs=6) as w3p, \
             tc.tile_pool(name="outp", bufs=4) as outp, \
             tc.tile_pool(name="psD", bufs=8, space="PSUM") as psD:

            # gh chunk loads only depend on phase-B stores; with bufs=8 the
            # first loads complete during phase B/C, so quantize fires the
            # moment rho3rep lands and mm3 ramps without DMA stalls.
            q3c = []
            for hc in range(HC):
                ghl = ghlp.tile([128, T_CORE], F32, tag="ghl")
                nc.scalar.dma_start(out=ghl,
                                    in_=gh_d[hc * 128:(hc + 1) * 128, :])
                nc.vector.tensor_tensor(out=ghl, in0=ghl, in1=rho3rep,
                                        op=mybir.AluOpType.mult)
                q3 = q3p.tile([128, T_CORE], BF16, tag=f"q3_{hc}")
                nc.vector.tensor_scalar(out=q3, in0=ghl, scalar1=C_RINT,
                                        scalar2=C_RINT,
                                        op0=mybir.AluOpType.add,
                                        op1=mybir.AluOpType.subtract)
                q3c.append(q3)

            for dq in range(DQ):
                dcol = dq * 512
                pos = [psD.tile([128, 512], F32, tag="po", name=f"po{dq}_{t}")
                       for t in range(TT)]
                for hc in range(HC):
                    w3b = w3p.tile([128, 512], BF16, tag="w3b")
                    nc.sync.dma_start(
                        out=w3b,
                        in_=w3_d[hc * 128:(hc + 1) * 128, dcol:dcol + 512])
                    for t in range(TT):
                        nc.tensor.matmul(
                            pos[t],
                            lhsT=q3c[hc][:, t * 128:(t + 1) * 128],
                            rhs=w3b,
                            start=(hc == 0), stop=(hc == HC - 1),
                            skip_group_check=True)
                for t in range(TT):
                    ob = outp.tile([128, 512], F32, tag="ob")
                    if t % 2 == 0:
                        nc.scalar.mul(out=ob, in_=pos[t], mul=c3_t[:, t:t + 1])
                    else:
                        nc.vector.tensor_scalar(out=ob, in0=pos[t],
                                                scalar1=c3_t[:, t:t + 1],
                                                scalar2=None,
                                                op0=mybir.AluOpType.mult)
                    nc.scalar.dma_start(
                        out=out_d[t * 128:(t + 1) * 128, dcol:dcol + 512],
                        in_=ob)

    nc.compile()
    return nc


_NC_CACHE = {}


def _get_program(g12_same=True, g_ones=True):
    key = (bool(g12_same), bool(g_ones))
    if key not in _NC_CACHE:
        _NC_CACHE[key] = _build_program(*key)
    return _NC_CACHE[key]


def _ternary_T(w):
    """Host ternarization matching round(tanh(w/(mean|w|+eps))) in value.
    Uses CPU-jax to replicate the reference's fp32 tanh bit-for-bit.
    Returns (transposed ternary bf16 array, arctanh(s) as float32)."""
    w32 = np.asarray(w, dtype=np.float32)
    try:
        import jax
        import jax.numpy as jnp
        cpu = jax.devices("cpu")[0]
        with jax.default_device(cpu):
            s = jnp.mean(jnp.abs(jnp.asarray(w32)))
            t = np.asarray(jnp.round(jnp.tanh(w32 / (s + np.float32(EPS)))))
            a = np.float32(jnp.arctanh(s))
    except Exception:
        s32 = np.float32(np.mean(np.abs(w32), dtype=np.float64))
        denom = np.float32(s32 + np.float32(EPS))
        thresh = np.float32(ATANH_HALF) * denom
        t = np.sign(w32) * (np.abs(w32) > thresh)
        a = np.float32(np.arctanh(np.float64(s32)))
    return np.ascontiguousarray(t.T).astype(ml_dtypes.bfloat16), a


def _gflags(g1, g2, g3):
    g1f = np.asarray(g1, np.float32).reshape(-1)
    g2f = np.asarray(g2, np.float32).reshape(-1)
    g3f = np.asarray(g3, np.float32).reshape(-1)
    g12_same = bool(np.array_equal(g1f, g2f))
    g_ones = bool((g1f == 1.0).all() and (g2f == 1.0).all()
                  and (g3f == 1.0).all())
    return g12_same, g_ones


def _make_inputs(x, w1, g1, w2, g2, w3, g3):
    x32 = np.asarray(x, np.float32).reshape(NTOK, D)
    w1q, a1 = _ternary_T(w1)            # [D, H]
    w2q, a2 = _ternary_T(w2)            # [D, H]
    w3q, a3 = _ternary_T(w3)            # [H, D] (w3 is [D, H])
    g1f = np.ascontiguousarray(np.asarray(g1, np.float32).reshape(1, D))
    g2f = np.ascontiguousarray(np.asarray(g2, np.float32).reshape(1, D))
    g3c = np.ascontiguousarray(
        np.asarray(g3, np.float32).reshape(HC, 128).T)   # [128, HC]
    kconst = np.array([[a1 / 127.0, a2 / 127.0, a3 / 127.0]], np.float32)
    in_maps = []
    for c in range(NCORES):
        in_maps.append({
            "x": np.ascontiguousarray(x32[c * T_CORE:(c + 1) * T_CORE]),
            "w1q": w1q, "w2q": w2q, "w3q": w3q,
            "g1": g1f, "g2": g2f, "g3c": g3c,
            "kconst": kconst,
        })
    return in_maps


def kernel(x, w1, g1, w2, g2, w3, g3):
    g12_same, g_ones = _gflags(g1, g2, g3)
    nc = _get_program(g12_same, g_ones)
    in_maps = _make_inputs(x, w1, g1, w2, g2, w3, g3)
    res = run_bass_kernel_spmd(nc, in_maps, list(range(NCORES)))
    out = np.concatenate([res.results[c]["out"] for c in range(NCORES)], axis=0)
    return out.reshape(B, S, D)
